# revision 1
# baseline (speedup 1.0000x reference)
"""EquivariantAttention kernel for 8 trn2 NeuronCores (Bass/Tile).

Strategy: shard edges by destination node (host sorts edges by dst).
Core c owns nodes [1250c, 1250(c+1)) and all edges pointing into them, so
edge-softmax and the scatter-sum are core-local (no collectives).

Device-side per 128-edge tile (edges on SBUF partitions):
  MLP1 on PE (features on partitions), then a *transposed* layer-2 matmul
  (lhsT = h-tile, rhs = W2^T) so rw lands [128 edges, 768] in PSUM.
  tmp/conv einsums + attention on DVE. Softmax max-subtraction is skipped
  (scores are bounded; exp/sum identical in exact arithmetic) and the
  per-edge division is folded into one per-node reciprocal:
    out[n] = segsum(ex*v)[n] / segsum(ex)[n].
  Segment sums via one-hot matmuls accumulated in PSUM per 128-node window.
"""

import time

import numpy as np

import concourse.bacc as bacc
import concourse.bass as bass
import concourse.mybir as mybir
import concourse.tile as tile
from concourse.bass_utils import run_bass_kernel_spmd

F32 = mybir.dt.float32
BF16 = mybir.dt.bfloat16
AF = mybir.ActivationFunctionType
USE_BF16 = False  # bf16 bilinear path is faster in the cost model but
# miscomputes on HW (layout/mode issue, unresolved); fp32 path is validated.

E = 160000
N = 10000
NC = 8
NPC = N // NC          # 1250 nodes per core
WIN = 128
NWIN = (NPC + WIN - 1) // WIN   # 10 windows per core
M1, M2, D1, D2, NREPS = 16, 8, 3, 3, 2
EDGE_DIM, HID, NHEADS = 32, 64, 4
HIDDEN = M2 * D2        # 24
HEAD = HIDDEN // NHEADS  # 6
TEMP = float(HIDDEN) ** (-0.5)

_CACHE = {}
LAST_RUN_S = None


def _build(T, toff):
    """Build the Bass program for T 128-edge tiles; toff[w] = first tile of
    window w (length NWIN+1)."""
    nc = bacc.Bacc(None, target_bir_lowering=False, debug=False)
    EP = T * 128
    ef_d = nc.dram_tensor("efT", [EDGE_DIM + 1, EP], F32, kind="ExternalInput")
    pk_d = nc.dram_tensor("packed", [T, 128, 84], F32, kind="ExternalInput")
    pkb_d = nc.dram_tensor("packedb", [T, 128, 88], BF16, kind="ExternalInput")
    w1_d = nc.dram_tensor("w1", [EDGE_DIM + 1, HID], F32, kind="ExternalInput")
    w2_d = nc.dram_tensor("w2", [HID + 1, 768], F32, kind="ExternalInput")
    io_d = nc.dram_tensor("iota", [128, 128], F32, kind="ExternalInput")
    out_d = nc.dram_tensor("out", [NWIN * 128, HIDDEN], F32, kind="ExternalOutput")

    with tile.TileContext(nc) as tc:
        with (
            tc.tile_pool(name="const", bufs=1) as cp,
            tc.tile_pool(name="sb", bufs=3) as pool,
            tc.tile_pool(name="ps", bufs=2, space="PSUM") as pp,
            tc.tile_pool(name="seg", bufs=2, space="PSUM") as sp,
        ):
            w1_sb = cp.tile([EDGE_DIM + 1, HID], F32)
            nc.sync.dma_start(w1_sb[:], w1_d[:])
            w2_sb = cp.tile([HID + 1, 768], F32)
            nc.sync.dma_start(w2_sb[:], w2_d[:])
            io_sb = cp.tile([128, 128], F32)
            nc.sync.dma_start(io_sb[:], io_d[:])
            # manual 3-deep rotation for h so the ones-row is set once
            h_bufs = [cp.tile([HID + 1, 128], F32, name=f"hbuf{i}") for i in range(3)]
            for hb in h_bufs:
                nc.vector.memset(hb[HID : HID + 1, :], 1.0)

            for w in range(NWIN):
                seg = sp.tile([128, 28], F32, tag="seg")
                t0, t1 = toff[w], toff[w + 1]
                for t in range(t0, t1):
                    ef_t = pool.tile([EDGE_DIM + 1, 128], F32, tag="ef")
                    nc.sync.dma_start(ef_t[:], ef_d[:, t * 128 : (t + 1) * 128])
                    pk_t = pool.tile([128, 84], F32, tag="pk")
                    nc.sync.dma_start(pk_t[:], pk_d[t])
                    if USE_BF16:
                        pkb_t = pool.tile([128, 88], BF16, tag="pkb")
                        nc.sync.dma_start(pkb_t[:], pkb_d[t])

                    h_ps = pp.tile([HID, 128], F32, tag="hps")
                    nc.tensor.matmul(h_ps[:], w1_sb[:], ef_t[:], start=True, stop=True)
                    h_sb = h_bufs[t % 3]
                    nc.scalar.activation(h_sb[0:HID, :], h_ps[:], AF.Relu)

                    conv_t = pool.tile([128, 72], F32, tag="conv")
                    if USE_BF16:
                        # one PSUM bank per tile; ACT casts each bank to bf16
                        rw_bf = pool.tile([128, 768], BF16, tag="rwbf")
                        rw_lo = pp.tile([128, 384], F32, tag="rwlo")
                        nc.tensor.matmul(
                            rw_lo[:], h_sb[:], w2_sb[:, 0:384], start=True, stop=True
                        )
                        nc.scalar.activation(rw_bf[:, 0:384], rw_lo[:], AF.Copy)
                        rw_hi = pp.tile([128, 384], F32, tag="rwhi")
                        nc.tensor.matmul(
                            rw_hi[:], h_sb[:], w2_sb[:, 384:768], start=True, stop=True
                        )
                        nc.scalar.activation(rw_bf[:, 384:768], rw_hi[:], AF.Copy)
                        # tmp product: fe[m,dp] * basisT[(dd,r),dp], dp padded to 4
                        fe4 = (
                            pkb_t[:, 0:64]
                            .rearrange("p (m d) -> p m d", d=4)
                            .unsqueeze(2)
                            .broadcast_to([128, M1, 6, 4])
                        )
                        bas4 = (
                            pkb_t[:, 64:88]
                            .rearrange("p (q d) -> p q d", d=4)
                            .unsqueeze(1)
                            .broadcast_to([128, M1, 6, 4])
                        )
                        prod = pool.tile([128, 384], BF16, tag="prod")
                        pv = prod[:].rearrange("p (m q d) -> p m q d", q=6, d=4)
                        nc.vector.tensor_mul(pv, fe4, bas4)
                        # reduce over d (3 real) per dd -> tmp fp32, dd-major
                        tmp_f = pool.tile([128, 96], F32, tag="tmpf")
                        pr = prod[:].rearrange(
                            "p (m dd r d) -> p m dd r d", dd=3, r=2, d=4
                        )
                        for dd in range(3):
                            nc.vector.tensor_reduce(
                                tmp_f[:, dd * 32 : dd * 32 + 32].rearrange(
                                    "p (m r) -> p m r", r=2
                                ),
                                pr[:, :, dd, :, 0:3],
                                axis=mybir.AxisListType.X,
                                op=mybir.AluOpType.add,
                            )
                        tmp_bf = pool.tile([128, 96], BF16, tag="tmpbf")
                        nc.vector.tensor_copy(tmp_bf[:], tmp_f[:])
                        # conv products in bf16 (2x mode), tree-reduce
                        rwv = (
                            rw_bf[:]
                            .rearrange("p (c j) -> p c j", j=32)
                            .unsqueeze(2)
                            .broadcast_to([128, 24, 3, 32])
                        )
                        tmpv = (
                            tmp_bf[:]
                            .rearrange("p (dd j) -> p dd j", j=32)
                            .unsqueeze(1)
                            .broadcast_to([128, 24, 3, 32])
                        )
                        pc = pool.tile([128, 2304], BF16, tag="pc")
                        pcv = pc[:].rearrange("p (c dd j) -> p c dd j", dd=3, j=32)
                        nc.vector.tensor_mul(pcv, rwv, tmpv)
                        lv = pc[:].rearrange("p (g j) -> p g j", j=32)
                        widths = [16, 8, 4, 2, 1]
                        cur = lv
                        for li, wdt in enumerate(widths):
                            if wdt == 1:
                                nxt_t = conv_t
                            else:
                                dt_l = BF16 if wdt > 4 else F32
                                nxt_t = pool.tile(
                                    [128, 72 * wdt], dt_l, tag=f"tr{li}", name=f"tr{li}"
                                )
                            nxt = nxt_t[:].rearrange("p (g j) -> p g j", j=wdt)
                            nc.vector.tensor_add(
                                nxt, cur[:, :, 0:wdt], cur[:, :, wdt : 2 * wdt]
                            )
                            cur = nxt
                    else:
                        # fp32 path (validated)
                        rw_lo = pp.tile([128, 384], F32, tag="rwlo")
                        nc.tensor.matmul(
                            rw_lo[:], h_sb[:], w2_sb[:, 0:384], start=True, stop=True
                        )
                        rw_hi = pp.tile([128, 384], F32, tag="rwhi")
                        nc.tensor.matmul(
                            rw_hi[:], h_sb[:], w2_sb[:, 384:768], start=True, stop=True
                        )
                        fe3 = pk_t[:, 0:64].rearrange("p (m d) -> p m d", d=4)[
                            :, :, 0:3
                        ]
                        fe4 = fe3.unsqueeze(2).broadcast_to([128, M1, 6, 3])
                        bas = pk_t[:, 64:82].rearrange("p (d q) -> p d q", q=6)
                        bas4 = bas.transpose([0, 2, 1]).unsqueeze(1).broadcast_to(
                            [128, M1, 6, 3]
                        )
                        prod = pool.tile([128, 288], F32, tag="prod")
                        pv = prod[:].rearrange("p (m q d) -> p m q d", q=6, d=3)
                        nc.vector.tensor_mul(pv, fe4, bas4)
                        tmp_t = pool.tile([128, 96], F32, tag="tmp")
                        nc.vector.tensor_reduce(
                            tmp_t[:].rearrange("p (m q) -> p m q", q=6),
                            pv,
                            axis=mybir.AxisListType.X,
                            op=mybir.AluOpType.add,
                        )
                        tmpv = (
                            tmp_t[:]
                            .rearrange("p (j dd) -> p j dd", dd=3)
                            .transpose([0, 2, 1])
                            .unsqueeze(1)
                            .broadcast_to([128, 12, 3, 32])
                        )
                        for half, rwb in ((0, rw_lo), (1, rw_hi)):
                            rwv = (
                                rwb[:]
                                .rearrange("p (c j) -> p c j", j=32)
                                .unsqueeze(2)
                                .broadcast_to([128, 12, 3, 32])
                            )
                            pc = pool.tile(
                                [128, 1152], F32, tag=f"pc{half}", name=f"pc{half}"
                            )
                            pcv = pc[:].rearrange("p (c dd j) -> p c dd j", dd=3, j=32)
                            nc.vector.tensor_mul(pcv, rwv, tmpv)
                            nc.vector.tensor_reduce(
                                conv_t[:, half * 36 : half * 36 + 36].rearrange(
                                    "p (c dd) -> p c dd", dd=3
                                ),
                                pcv,
                                axis=mybir.AxisListType.X,
                                op=mybir.AluOpType.add,
                            )

                    # scores -> leaky relu -> exp  (temp folded into W2 on host)
                    p4 = pool.tile([128, 24], F32, tag="p4")
                    nc.vector.tensor_mul(p4[:], conv_t[:, 0:24], conv_t[:, 24:48])
                    s4 = pool.tile([128, 4], F32, tag="s4")
                    nc.vector.tensor_reduce(
                        s4[:],
                        p4[:].rearrange("p (h j) -> p h j", j=6),
                        axis=mybir.AxisListType.X,
                        op=mybir.AluOpType.add,
                    )
                    t4 = pool.tile([128, 4], F32, tag="t4")
                    nc.vector.tensor_scalar_mul(t4[:], s4[:], 0.2)
                    l4 = pool.tile([128, 4], F32, tag="l4")
                    nc.vector.tensor_max(l4[:], s4[:], t4[:])
                    x_t = pool.tile([128, 28], F32, tag="xt")
                    nc.scalar.activation(x_t[:, 0:4], l4[:], AF.Exp)
                    exb = x_t[:, 0:4].unsqueeze(2).broadcast_to([128, 4, 6])
                    nc.vector.tensor_mul(
                        x_t[:, 4:28].rearrange("p (h j) -> p h j", j=6),
                        conv_t[:, 48:72].rearrange("p (h j) -> p h j", j=6),
                        exb,
                    )

                    # one-hot over the window's 128 node slots; -1 rows -> all 0
                    oh = pool.tile([128, 128], F32, tag="oh")
                    nc.vector.tensor_tensor(
                        oh[:],
                        pk_t[:, 82:83].broadcast_to([128, 128]),
                        io_sb[:],
                        op=mybir.AluOpType.is_equal,
                    )
                    nc.tensor.matmul(
                        seg[:],
                        oh[:],
                        x_t[:],
                        start=(t == t0),
                        stop=(t == t1 - 1),
                        skip_group_check=True,
                    )

                # flush window: out = num / den
                den = pool.tile([128, 4], F32, tag="den")
                nc.vector.tensor_scalar_add(den[:], seg[:, 0:4], 1e-30)
                rcp = pool.tile([128, 4], F32, tag="rcp")
                nc.vector.reciprocal(rcp[:], den[:])
                outw = pool.tile([128, HIDDEN], F32, tag="outw")
                nc.vector.tensor_mul(
                    outw[:].rearrange("p (h j) -> p h j", j=6),
                    seg[:, 4:28].rearrange("p (h j) -> p h j", j=6),
                    rcp[:].unsqueeze(2).broadcast_to([128, 4, 6]),
                )
                nc.sync.dma_start(out_d[w * 128 : (w + 1) * 128, :], outw[:])
    nc.finalize()
    return nc


def _prep(src, dst, basis, edge_feats, f, W1, b1, W2, b2):
    src = np.asarray(src).astype(np.int64)
    dst = np.asarray(dst).astype(np.int64)
    basis = np.asarray(basis, dtype=np.float32)
    edge_feats = np.asarray(edge_feats, dtype=np.float32)
    f = np.asarray(f, dtype=np.float32)

    order = np.argsort(dst, kind="stable")
    ds = dst[order]
    # window boundaries: global windows = core*NWIN + w, node range below
    starts = []
    for c in range(NC):
        for w in range(NWIN):
            starts.append(c * NPC + w * WIN)
    starts.append(N)
    cuts = np.searchsorted(ds, np.array(starts))
    cnt = cuts[1:] - cuts[:-1]  # edges per (core,window), len NC*NWIN
    cnt = cnt.reshape(NC, NWIN)
    tw = np.maximum(1, (cnt + 127) // 128).max(axis=0)  # tiles per window
    toff = np.zeros(NWIN + 1, dtype=np.int64)
    toff[1:] = np.cumsum(tw)
    T = int(toff[-1])
    EP = T * 128

    # shared tensors
    s = np.ones(768, dtype=np.float32)
    s[: 16 * 32] = TEMP**0.5  # k and q blocks carry sqrt(temp) each
    w1_aug = np.concatenate(
        [np.asarray(W1, dtype=np.float32).T, np.asarray(b1, dtype=np.float32)[None, :]]
    )  # [33, 64]
    w2_aug = np.concatenate(
        [
            np.asarray(W2, dtype=np.float32).T * s[None, :],
            (np.asarray(b2, dtype=np.float32) * s)[None, :],
        ]
    )  # [65, 768]
    iota = np.broadcast_to(np.arange(128, dtype=np.float32)[None, :], (128, 128)).copy()

    in_maps = []
    for c in range(NC):
        efT = np.zeros((EDGE_DIM + 1, EP), dtype=np.float32)
        packed = np.zeros((T, 128, 84), dtype=np.float32)
        packedb = np.zeros((T, 128, 88), dtype=np.float32)
        packed[:, :, 82] = -1.0
        for w in range(NWIN):
            a, b = cuts[c * NWIN + w], cuts[c * NWIN + w + 1]
            idx = order[a:b]
            k = len(idx)
            if k == 0:
                continue
            base = toff[w] * 128
            efT[:EDGE_DIM, base : base + k] = edge_feats[idx].T
            efT[EDGE_DIM, base : base + k] = 1.0
            fe = f[src[idx]]  # [k, 16, 3]
            fep = np.zeros((k, M1, 4), dtype=np.float32)
            fep[:, :, :3] = fe
            flat = packed.reshape(T * 128, 84)
            flat[base : base + k, 0:64] = fep.reshape(k, 64)
            flat[base : base + k, 64:82] = basis[idx].reshape(k, 18)
            flat[base : base + k, 82] = (dst[idx] - c * NPC - w * WIN).astype(
                np.float32
            )
            flatb = packedb.reshape(T * 128, 88)
            flatb[base : base + k, 0:64] = fep.reshape(k, 64)
            # basis as [(dd,r), dpad4]: basisT[ddr, d] = basis[d, r*3+dd]
            bt = basis[idx].reshape(k, 3, 2, 3)  # (d, r, dd)
            btp = np.zeros((k, 3, 2, 4), dtype=np.float32)
            btp[:, :, :, :3] = bt.transpose(0, 3, 2, 1)  # (dd, r, d)
            flatb[base : base + k, 64:88] = btp.reshape(k, 24)
        import ml_dtypes

        in_maps.append(
            {
                "efT": efT,
                "packed": packed,
                "packedb": packedb.astype(ml_dtypes.bfloat16),
                "w1": w1_aug,
                "w2": w2_aug,
                "iota": iota,
            }
        )
    return T, toff, in_maps


def kernel(src, dst, basis, edge_feats, f, W1, b1, W2, b2):
    global LAST_RUN_S
    T, toff, in_maps = _prep(src, dst, basis, edge_feats, f, W1, b1, W2, b2)
    key = (T, tuple(toff))
    if key not in _CACHE:
        _CACHE[key] = _build(T, toff)
    nc = _CACHE[key]
    t0 = time.time()
    import os

    trace = bool(os.environ.get("BASS_KTRACE"))
    res = run_bass_kernel_spmd(nc, in_maps, list(range(NC)), trace=trace)
    LAST_RUN_S = time.time() - t0
    global LAST_RESULTS
    LAST_RESULTS = res
    outs = [res.results[c]["out"][:NPC] for c in range(NC)]
    full = np.concatenate(outs, axis=0).astype(np.float32)
    return full.reshape(N, M2, D2)



# revision 5
# speedup vs baseline: 1.7574x; 1.7574x over previous
"""EquivariantAttention kernel for 8 trn2 NeuronCores (Bass/Tile).

Strategy: shard edges by destination node (host sorts edges by dst).
Core c owns nodes [1250c, 1250(c+1)) and all edges pointing into them, so
edge-softmax and the scatter-sum are core-local (no collectives).

Device-side per 512-edge tile (4 edge "slots" x 128 partitions):
  MLP in fp16 on PE (features on partitions); rw lands [128, 768] per slot
  in PSUM and is evicted to fp16 SBUF by the Activation engine.
  tmp/conv einsums run on DVE in fp16 (2-byte dtypes get the DVE 2x mode);
  the j=32 contraction is a tree of tensor-tensor adds.  Softmax max-
  subtraction is skipped (scores bounded); leaky-relu+exp is fused as
  exp(leaky(x)) = max(exp(x), exp(0.2x)) with both exps on ACT.
  Per-edge division is folded into one per-node reciprocal:
    out[n] = segsum(ex*v)[n] / segsum(ex)[n].
  Segment sums via one-hot matmuls accumulated in PSUM per 128-node window.
"""

import os
import time

import numpy as np

import concourse.bacc as bacc
import concourse.bass as bass
import concourse.mybir as mybir
import concourse.tile as tile
from concourse.bass_utils import run_bass_kernel_spmd

F32 = mybir.dt.float32
F16 = mybir.dt.float16
BF16 = mybir.dt.bfloat16
AF = mybir.ActivationFunctionType
ALU = mybir.AluOpType

E = 160000
N = 10000
NC = 8
NPC = N // NC          # 1250 nodes per core
WIN = 128
NWIN = (NPC + WIN - 1) // WIN   # 10 windows per core
M1, M2, D1, D2, NREPS = 16, 8, 3, 3, 2
EDGE_DIM, HID, NHEADS = 32, 64, 4
HIDDEN = M2 * D2        # 24
TEMP = float(HIDDEN) ** (-0.5)

NS = 4                  # edge slots per partition
TS = NS * 128           # 512 edges per tile
PKC = 68                # pk cols per slot: fe 48 + basis 18 + dst 1 + pad 1

_CACHE = {}
LAST_RUN_S = None
LAST_RESULTS = None


def _build(T, toff):
    """T 512-edge tiles; toff[w] = first tile of window w (length NWIN+1)."""
    nc = bacc.Bacc(None, target_bir_lowering=False, debug=False)
    EP = T * TS
    ef_d = nc.dram_tensor("efT", [EDGE_DIM + 1, EP], F16, kind="ExternalInput")
    pk_d = nc.dram_tensor("pk", [T, 128, NS * PKC], F16, kind="ExternalInput")
    w1_d = nc.dram_tensor("w1", [EDGE_DIM + 1, HID], F16, kind="ExternalInput")
    w2_d = nc.dram_tensor("w2", [HID + 1, 768], F16, kind="ExternalInput")
    io_d = nc.dram_tensor("iota", [128, 128], BF16, kind="ExternalInput")
    out_d = nc.dram_tensor("out", [NWIN * 128, HIDDEN], F32, kind="ExternalOutput")

    with tile.TileContext(nc) as tc:
        with (
            tc.tile_pool(name="const", bufs=1) as cp,
            tc.tile_pool(name="sb", bufs=2) as pool,
            tc.tile_pool(name="hp", bufs=2, space="PSUM") as hp,
            tc.tile_pool(name="rwp", bufs=4, space="PSUM") as rwp,
            tc.tile_pool(name="seg", bufs=2, space="PSUM") as sp,
        ):
            w1_sb = cp.tile([EDGE_DIM + 1, HID], F16)
            nc.sync.dma_start(w1_sb[:], w1_d[:])
            w2_sb = cp.tile([HID + 1, 768], F16)
            nc.sync.dma_start(w2_sb[:], w2_d[:])
            io_sb = cp.tile([128, 128], BF16)
            nc.sync.dma_start(io_sb[:], io_d[:])
            # manual rotation for h so the ones-row is set once
            h_bufs = [cp.tile([HID + 1, TS], F16, name=f"hbuf{i}") for i in range(2)]
            for hb in h_bufs:
                nc.vector.memset(hb[HID : HID + 1, :], 1.0)

            for w in range(NWIN):
                seg = sp.tile([128, 28], F32, tag="seg")
                t0, t1 = toff[w], toff[w + 1]
                for t in range(t0, t1):
                    ef_t = pool.tile([EDGE_DIM + 1, TS], F16, tag="ef")
                    nc.sync.dma_start(ef_t[:], ef_d[:, t * TS : (t + 1) * TS])
                    pk_t = pool.tile([128, NS * PKC], F16, tag="pk")
                    nc.sync.dma_start(pk_t[:], pk_d[t])

                    # ---- MLP1: h = relu(W1 @ ef + b1), features on partitions
                    h_ps = hp.tile([HID, TS], F32, tag="hps")
                    nc.tensor.matmul(h_ps[:], w1_sb[:], ef_t[:], start=True, stop=True)
                    h_sb = h_bufs[t % 2]
                    nc.scalar.activation(h_sb[0:HID, :], h_ps[:], AF.Relu)

                    # ---- MLP2 per slot: rw[e, (c,m,r)] in PSUM, evict to fp16
                    rw_sb = pool.tile([128, NS * 768], F16, tag="rwsb")
                    for s in range(NS):
                        for half in range(2):
                            rw_ps = rwp.tile([128, 384], F32, tag="rw")
                            nc.tensor.matmul(
                                rw_ps[:],
                                h_sb[:, s * 128 : (s + 1) * 128],
                                w2_sb[:, half * 384 : (half + 1) * 384],
                                start=True,
                                stop=True,
                            )
                            nc.scalar.activation(
                                rw_sb[:, s * 768 + half * 384 : s * 768 + (half + 1) * 384],
                                rw_ps[:],
                                AF.Copy,
                            )

                    # ---- tmp[e, dd, (m,r)] = sum_p fe[e,m,p] * basis[e,(dd,r),p]
                    # pk layout per slot: fe [m(16), p(3)] then basis [dd(3), r(2), p(3)]
                    # (HW ISA allows max 3 free dims per DVE operand -> loop slots)
                    tp = pool.tile([128, NS * 288], F16, tag="tp")
                    tm = pool.tile([128, NS * 96], F16, tag="tm")
                    tmA = pool.tile([128, NS * 96], F16, tag="tmA")
                    for s in range(NS):
                        fe = pk_t[:, s * PKC : s * PKC + 48]
                        bas = pk_t[:, s * PKC + 48 : s * PKC + 66]
                        fe_v = (
                            fe.rearrange("p (m d) -> p m d", d=3)
                            .unsqueeze(2)
                            .broadcast_to([128, M1, 6, 3])
                        )
                        bas_v = (
                            bas.rearrange("p (q d) -> p q d", d=3)
                            .unsqueeze(1)
                            .broadcast_to([128, M1, 6, 3])
                        )
                        tp_s = tp[:, s * 288 : (s + 1) * 288]
                        nc.vector.tensor_mul(
                            tp_s.rearrange("p (m q d) -> p m q d", q=6, d=3),
                            fe_v,
                            bas_v,
                        )
                        # sum over p (strided, 1x) writing transposed to [dd, m, r]
                        tpq = tp_s.rearrange(
                            "p (m dd r d) -> p m dd r d", dd=3, r=2, d=3
                        )
                        tmA_s = tmA[:, s * 96 : (s + 1) * 96]
                        tm_s = tm[:, s * 96 : (s + 1) * 96]
                        tmA_o = tmA_s.rearrange("p (dd m r) -> p m dd r", dd=3, r=2)
                        tm_o = tm_s.rearrange("p (dd m r) -> p m dd r", dd=3, r=2)
                        nc.vector.tensor_add(
                            tmA_o, tpq[:, :, :, :, 0], tpq[:, :, :, :, 1]
                        )
                        tmA_i = tmA_s.rearrange("p (dd m r) -> p m dd r", dd=3, r=2)
                        nc.vector.tensor_add(tm_o, tmA_i, tpq[:, :, :, :, 2])

                    # ---- conv products [c, dd, j] per slot; tree over j=32 fused
                    pc = pool.tile([128, NS * 2304], F16, tag="pc")
                    for s in range(NS):
                        rw_v = (
                            rw_sb[:, s * 768 : (s + 1) * 768]
                            .rearrange("p (c j) -> p c j", j=32)
                            .unsqueeze(2)
                            .broadcast_to([128, 24, 3, 32])
                        )
                        tm_v = (
                            tm[:, s * 96 : (s + 1) * 96]
                            .rearrange("p (dd j) -> p dd j", j=32)
                            .unsqueeze(1)
                            .broadcast_to([128, 24, 3, 32])
                        )
                        nc.vector.tensor_mul(
                            pc[:, s * 2304 : (s + 1) * 2304].rearrange(
                                "p (c dd j) -> p c dd j", dd=3, j=32
                            ),
                            rw_v,
                            tm_v,
                        )

                    conv_t = pool.tile([128, NS * 72], F32, tag="conv")
                    cur = pc[:].rearrange("p (s g j) -> p s g j", g=72, j=32)
                    for li, wdt in enumerate([16, 8, 4, 2, 1]):
                        if wdt == 1:
                            nxt_t = conv_t
                        else:
                            nxt_t = pool.tile(
                                [128, NS * 72 * wdt], F16, tag=f"tr{li}", name=f"tr{li}"
                            )
                        nxt = nxt_t[:].rearrange("p (s g j) -> p s g j", g=72, j=wdt)
                        nc.vector.tensor_add(
                            nxt, cur[:, :, :, 0:wdt], cur[:, :, :, wdt : 2 * wdt]
                        )
                        cur = nxt

                    # ---- scores: s4 = sum_hd k*q  (temp folded into W2 on host)
                    cg = conv_t[:].rearrange("p (s g) -> p s g", g=72)
                    p4 = pool.tile([128, NS * 24], F32, tag="p4")
                    p4_v = p4[:].rearrange("p (s h x) -> p s h x", h=4, x=6)
                    nc.vector.tensor_mul(
                        p4_v,
                        cg[:, :, 0:24].rearrange("p s (h x) -> p s h x", x=6),
                        cg[:, :, 24:48].rearrange("p s (h x) -> p s h x", x=6),
                    )
                    s4 = pool.tile([128, NS * 4], F32, tag="s4")
                    nc.vector.tensor_reduce(
                        s4[:].rearrange("p (s h) -> p s h", h=4).unsqueeze(3),
                        p4_v,
                        axis=mybir.AxisListType.X,
                        op=ALU.add,
                    )
                    # exp(leaky(x)) = max(exp(x), exp(0.2x)); both exps on ACT
                    ea = pool.tile([128, NS * 4], BF16, tag="ea")
                    nc.scalar.activation(ea[:], s4[:], AF.Exp)
                    eb = pool.tile([128, NS * 4], BF16, tag="eb")
                    nc.scalar.activation(eb[:], s4[:], AF.Exp, scale=0.2)

                    x_t = pool.tile([128, NS * 28], BF16, tag="xt")
                    xs = x_t[:].rearrange("p (s c) -> p s c", c=28)
                    nc.vector.tensor_max(
                        xs[:, :, 0:4],
                        ea[:].rearrange("p (s h) -> p s h", h=4),
                        eb[:].rearrange("p (s h) -> p s h", h=4),
                    )
                    # wv = ex * v   [s, h, c2, dd] with ex broadcast over (c2,dd)
                    exv = (
                        x_t[:]
                        .rearrange("p (s c) -> p s c", c=28)[:, :, 0:4]
                        .unsqueeze(3)
                        .broadcast_to([128, NS, 4, 6])
                    )
                    nc.vector.tensor_mul(
                        xs[:, :, 4:28].rearrange("p s (h x) -> p s h x", x=6),
                        cg[:, :, 48:72].rearrange("p s (h x) -> p s h x", x=6),
                        exv,
                    )

                    # ---- one-hot over window's 128 node slots; -1 rows -> 0
                    dst = pk_t[:].rearrange("p (s c) -> p s c", c=PKC)[:, :, 66:67]
                    oh = pool.tile([128, NS * 128], BF16, tag="oh")
                    nc.vector.tensor_tensor(
                        oh[:].rearrange("p (s n) -> p s n", n=128),
                        dst.broadcast_to([128, NS, 128]),
                        io_sb[:].unsqueeze(1).broadcast_to([128, NS, 128]),
                        op=ALU.is_equal,
                    )
                    for s in range(NS):
                        nc.tensor.matmul(
                            seg[:],
                            oh[:, s * 128 : (s + 1) * 128],
                            x_t[:, s * 28 : (s + 1) * 28],
                            start=(t == t0 and s == 0),
                            stop=(t == t1 - 1 and s == NS - 1),
                            skip_group_check=True,
                        )

                # ---- flush window: out = num / den
                den = pool.tile([128, 4], F32, tag="den")
                nc.vector.tensor_scalar_add(den[:], seg[:, 0:4], 1e-30)
                rcp = pool.tile([128, 4], F32, tag="rcp")
                nc.vector.reciprocal(rcp[:], den[:])
                outw = pool.tile([128, HIDDEN], F32, tag="outw")
                nc.vector.tensor_mul(
                    outw[:].rearrange("p (h j) -> p h j", j=6),
                    seg[:, 4:28].rearrange("p (h j) -> p h j", j=6),
                    rcp[:].unsqueeze(2).broadcast_to([128, 4, 6]),
                )
                nc.sync.dma_start(out_d[w * 128 : (w + 1) * 128, :], outw[:])
    nc.finalize()
    return nc


NGW = (N + WIN - 1) // WIN  # 79 global windows of 128 nodes


def _prep(src, dst, basis, edge_feats, f, W1, b1, W2, b2):
    import ml_dtypes

    src = np.asarray(src).astype(np.int64)
    dst = np.asarray(dst).astype(np.int64)
    basis = np.asarray(basis, dtype=np.float32)
    edge_feats = np.asarray(edge_feats, dtype=np.float32)
    f = np.asarray(f, dtype=np.float32)

    order = np.argsort(dst, kind="stable")
    ds = dst[order]
    # global 128-node windows; edges of window g are the contiguous range
    cuts = np.searchsorted(ds, np.arange(0, NGW * WIN + 1, WIN).clip(max=N))
    cnt = cuts[1:] - cuts[:-1]  # len NGW
    # assign windows to (core, slot): sort desc, band s -> 8 cores at slot s.
    # All cores share one SPMD program, so slot s costs max tiles in its band.
    rank = np.argsort(-cnt, kind="stable")
    assign = -np.ones((NC, NWIN), dtype=np.int64)  # -1 = dummy window
    tw = np.zeros(NWIN, dtype=np.int64)
    for s in range(NWIN):
        band = rank[s * NC : (s + 1) * NC]
        for c, g in enumerate(band):
            assign[c, s] = g
        bmax = cnt[band].max() if len(band) else 0
        tw[s] = max(1, -(-int(bmax) // TS))
    toff = np.zeros(NWIN + 1, dtype=np.int64)
    toff[1:] = np.cumsum(tw)
    T = int(toff[-1])
    EP = T * TS

    sc = np.ones(768, dtype=np.float32)
    sc[: 16 * 32] = TEMP**0.5  # k and q blocks carry sqrt(temp) each
    w1_aug = np.concatenate(
        [np.asarray(W1, dtype=np.float32).T, np.asarray(b1, dtype=np.float32)[None, :]]
    ).astype(np.float16)  # [33, 64]
    w2_aug = np.concatenate(
        [
            np.asarray(W2, dtype=np.float32).T * sc[None, :],
            (np.asarray(b2, dtype=np.float32) * sc)[None, :],
        ]
    ).astype(np.float16)  # [65, 768]
    iota = (
        np.broadcast_to(np.arange(128, dtype=np.float32)[None, :], (128, 128))
        .astype(ml_dtypes.bfloat16)
        .copy()
    )

    # basis reordered per edge to [dd, r, p]: basis is (E, p, (r, dd))
    bas_ddrp = basis.reshape(E, 3, 2, 3).transpose(0, 3, 2, 1)  # (E, dd, r, p)

    in_maps = []
    for c in range(NC):
        efT = np.zeros((EDGE_DIM + 1, EP), dtype=np.float16)
        pk = np.zeros((T, 128, NS, PKC), dtype=np.float16)
        pk[:, :, :, 66] = -1.0
        for s in range(NWIN):
            g = assign[c, s]
            if g < 0:
                continue
            a, b = cuts[g], cuts[g + 1]
            idx = order[a:b]
            k = len(idx)
            if k == 0:
                continue
            base = toff[s] * TS
            efT[:EDGE_DIM, base : base + k] = edge_feats[idx].T
            efT[EDGE_DIM, base : base + k] = 1.0
            # edge i (within window) -> tile toff[s]+i//TS, slot, partition
            ti = (np.arange(k) // TS) + toff[s]
            sl = (np.arange(k) % TS) // 128
            pt = np.arange(k) % 128
            pk[ti, pt, sl, 0:48] = f[src[idx]].reshape(k, 48)
            pk[ti, pt, sl, 48:66] = bas_ddrp[idx].reshape(k, 18)
            pk[ti, pt, sl, 66] = (dst[idx] - g * WIN).astype(np.float32)
        in_maps.append(
            {
                "efT": efT,
                "pk": pk.reshape(T, 128, NS * PKC),
                "w1": w1_aug,
                "w2": w2_aug,
                "iota": iota,
            }
        )
    return T, toff, assign, in_maps


def kernel(src, dst, basis, edge_feats, f, W1, b1, W2, b2):
    global LAST_RUN_S, LAST_RESULTS
    T, toff, assign, in_maps = _prep(src, dst, basis, edge_feats, f, W1, b1, W2, b2)
    key = (T, tuple(toff))
    if key not in _CACHE:
        _CACHE[key] = _build(T, toff)
    nc = _CACHE[key]
    t0 = time.time()
    trace = bool(os.environ.get("BASS_KTRACE"))
    res = run_bass_kernel_spmd(nc, in_maps, list(range(NC)), trace=trace)
    LAST_RUN_S = time.time() - t0
    LAST_RESULTS = res
    full = np.zeros((N, HIDDEN), dtype=np.float32)
    for c in range(NC):
        o = np.asarray(res.results[c]["out"], dtype=np.float32)  # [NWIN*128, 24]
        for s in range(NWIN):
            g = assign[c, s]
            if g < 0:
                continue
            lo = g * WIN
            hi = min(lo + WIN, N)
            full[lo:hi] = o[s * WIN : s * WIN + (hi - lo)]
    return full.reshape(N, M2, D2)
